# revision 4
# baseline (speedup 1.0000x reference)
"""Sliding-window attention (B=2,T=2048,C=1024,H=16,HD=64,WINDOW=524) on 8 trn2 cores.

Sharding: sequence-parallel. Core k = b*4+c owns query rows [c*512,(c+1)*512) of
batch b and receives x rows [c*512-524, c*512+512) (zero-padded outside the
sequence) so all of its attention windows are local. No collectives.

Per-core device pipeline (everything transposed so contractions land on the
partition axis):
  qT = Wq^T x^T (only own 512 rows), kT = Wk^T x^T (all 1152 local rows),
  RoPE folded into elementwise cos/sin scaling:
     scores q'.k' == (q * cs_i) . (2k * cs_j)  with cs = [cos;sin] per head,
  V computed in natural (t, c) layout with a ones column appended,
  S^T blocks (kv on partitions, q on free dim) via f32r matmuls,
  P = exp(S/8) in bf16, multiplied by a precomputed band/edge mask,
  y^T (+ softmax denominator) = [V|1]^T @ P accumulated in PSUM,
  normalize by the reciprocal denominator row, output projection Wo^T y^T.
"""

import os
import sys

import numpy as np

for _p in ("/opt/trn_rl_repo",):
    if _p not in sys.path and os.path.isdir(_p):
        sys.path.insert(0, _p)

import ml_dtypes

import concourse.bacc as bacc
import concourse.bass as bass
import concourse.mybir as mybir
from concourse.bass_utils import run_bass_kernel_spmd
from concourse.tile import TileContext

F32 = mybir.dt.float32
F32R = mybir.dt.float32r
BF16 = mybir.dt.bfloat16

B, T, C, H, HD = 2, 2048, 1024, 16, 64
WINDOW = 524
P = 128
CH = 512          # query rows per core
HALO = 524        # kv halo rows before the chunk
KT = 1152         # padded local kv length (1036 -> 9*128)
NJ = KT // P      # 9 j-chunks
NCC = C // P      # 8 contraction chunks
NCORE = 8

# Per-j-chunk query windows [lo, hi) in local query coords, 16-aligned lo.
JW = []
_off = 0
for _jc in range(NJ):
    _lo = max(0, P * _jc - 528)
    _hi = min(CH, P * _jc + P)
    JW.append((_lo, _hi, _off))
    _off += _hi - _lo
MW = _off  # 2624

# k spans for the kT/vT projections (all widths >= 256 for f32r full rate)
KSPANS = [(0, 512), (512, 896), (896, 1152)]


def build_nc():
    nc = bacc.Bacc(None, target_bir_lowering=False)

    xT_d = nc.declare_dram_parameter("xT", [C, KT], F32R, isOutput=False)
    wqr_d = nc.declare_dram_parameter("wqr", [NCC, P, NCC, P], F32R, isOutput=False)
    wkr_d = nc.declare_dram_parameter("wkr", [NCC, P, NCC, P], F32R, isOutput=False)
    wor_d = nc.declare_dram_parameter("wor", [NCC, P, NCC, P], F32R, isOutput=False)
    wvr_d = nc.declare_dram_parameter("wvr", [NCC, P, C], F32R, isOutput=False)
    csq_d = nc.declare_dram_parameter("csq", [P, CH], F32, isOutput=False)
    csk_d = nc.declare_dram_parameter("csk", [P, KT], F32, isOutput=False)
    mask_d = nc.declare_dram_parameter("mask", [P, MW], BF16, isOutput=False)
    out_d = nc.declare_dram_parameter("out", [C, CH], F32, isOutput=True)

    Exp = mybir.ActivationFunctionType.Exp

    with TileContext(nc) as tc:
        with tc.tile_pool(name="persist", bufs=1) as pers:
            csq_sb = pers.tile([P, CH], F32, name="csq_sb")
            nc.sync.dma_start(csq_sb[:], csq_d[:])
            csk_sb = pers.tile([P, KT], F32, name="csk_sb")
            nc.sync.dma_start(csk_sb[:], csk_d[:])
            mask_sb = pers.tile([P, MW], BF16, name="mask_sb")
            nc.sync.dma_start(mask_sb[:], mask_d[:])

            qt_sb = []
            kt_sb = []
            yt_sb = []
            for cc2 in range(NCC):
                qt_sb.append(pers.tile([P, CH], F32R, name=f"qt{cc2}"))
                kt_sb.append(pers.tile([P, KT], F32R, name=f"kt{cc2}"))
                yt_sb.append(pers.tile([P, CH], F32R, name=f"yt{cc2}"))
            v_sb = pers.tile([P, NJ, H, 65], BF16, name="v_sb")
            nc.vector.memset(v_sb[:, :, :, 64], 1.0)

            # ---------------- Phase B: projections ----------------
            with (
                tc.tile_pool(name="xw", bufs=1) as xw,
                tc.tile_pool(name="wv_pool", bufs=1) as wvp,
                tc.tile_pool(name="ws", bufs=3) as ws,
                tc.tile_pool(name="psb", bufs=2, space="PSUM") as psb,
                tc.tile_pool(name="psv_pool", bufs=2, space="PSUM") as psvp,
            ):
                xT_sb = []
                for cc in range(NCC):
                    xt = xw.tile([P, KT], F32R, name=f"xt{cc}")
                    nc.sync.dma_start(xt[:], xT_d[cc * P:(cc + 1) * P, :])
                    xT_sb.append(xt)
                wv_sb = []
                for cc in range(NCC):
                    wvt = wvp.tile([P, C], F32R, name=f"wvt{cc}")
                    nc.sync.dma_start(wvt[:], wvr_d[cc])
                    wv_sb.append(wvt)

                for cc2 in range(NCC):
                    wq_t = ws.tile([P, NCC, P], F32R, name="wq_t")
                    nc.sync.dma_start(wq_t[:], wqr_d[cc2])
                    wk_t = ws.tile([P, NCC, P], F32R, name="wk_t")
                    nc.sync.dma_start(wk_t[:], wkr_d[cc2])

                    psq = psb.tile([P, CH], F32, name="psq")
                    for cc in range(NCC):
                        nc.tensor.matmul(
                            psq[:],
                            lhsT=wq_t[:, cc, :],
                            rhs=xT_sb[cc][:, HALO:HALO + CH],
                            start=(cc == 0), stop=(cc == NCC - 1),
                        )
                    nc.vector.tensor_mul(qt_sb[cc2][:], psq[:], csq_sb[:])

                    for (a, b) in KSPANS:
                        psk = psb.tile([P, 512], F32, name="psk")
                        for cc in range(NCC):
                            nc.tensor.matmul(
                                psk[:, :b - a],
                                lhsT=wk_t[:, cc, :],
                                rhs=xT_sb[cc][:, a:b],
                                start=(cc == 0), stop=(cc == NCC - 1),
                            )
                        nc.vector.tensor_mul(
                            kt_sb[cc2][:, a:b], psk[:, :b - a], csk_sb[:, a:b]
                        )

                # V in natural layout: lhsT = xT tile (c, t-block), rhs = Wv rows
                for tb in range(NJ):
                    for half in range(2):
                        psv = psvp.tile([P, 512], F32, name="psv")
                        for cc in range(NCC):
                            nc.tensor.matmul(
                                psv[:],
                                lhsT=xT_sb[cc][:, tb * P:(tb + 1) * P],
                                rhs=wv_sb[cc][:, half * 512:(half + 1) * 512],
                                start=(cc == 0), stop=(cc == NCC - 1),
                            )
                        nc.scalar.copy(
                            v_sb[:, tb, half * 8:(half + 1) * 8, 0:64],
                            psv[:].rearrange("p (h d) -> p h d", h=8),
                        )

            # ---------------- Phase C: attention ----------------
            with (
                tc.tile_pool(name="ppool", bufs=2) as ppool,
                tc.tile_pool(name="rpool", bufs=2) as rpool,
                tc.tile_pool(name="pst_pool", bufs=3, space="PSUM") as pstp,
                tc.tile_pool(name="pay_pool", bufs=2, space="PSUM") as payp,
            ):
                jc_order = [3] + [j for j in range(NJ) if j != 3]
                for h in range(H):
                    cc2, po = h // 2, (h % 2) * 64
                    ps_y = payp.tile([P, CH], F32, name="ps_y")
                    P_t = ppool.tile([P, MW], BF16, name="P_t")
                    for idx, jc in enumerate(jc_order):
                        lo, hi, off = JW[jc]
                        w = hi - lo
                        ps_st = pstp.tile([P, 512], F32, name="ps_st")
                        nc.tensor.matmul(
                            ps_st[:, :w],
                            lhsT=kt_sb[cc2][po:po + 64, jc * P:(jc + 1) * P],
                            rhs=qt_sb[cc2][po:po + 64, lo:hi],
                            start=True, stop=True,
                        )
                        nc.scalar.activation(
                            P_t[:, off:off + w], ps_st[:, :w], Exp, scale=0.125
                        )
                        nc.vector.tensor_mul(
                            P_t[:, off:off + w], P_t[:, off:off + w],
                            mask_sb[:, off:off + w],
                        )
                        nc.tensor.matmul(
                            ps_y[0:65, lo:hi],
                            lhsT=v_sb[:, jc, h, :],
                            rhs=P_t[:, off:off + w],
                            start=(idx == 0), stop=(idx == NJ - 1),
                        )
                    rcp = rpool.tile([1, CH], F32, name="rcp")
                    nc.vector.reciprocal(rcp[:], ps_y[64:65, :])
                    rcp_bc = rpool.tile([64, CH], F32, name="rcp_bc")
                    nc.gpsimd.partition_broadcast(rcp_bc[:], rcp[:])
                    nc.vector.tensor_mul(
                        yt_sb[cc2][po:po + 64, :], ps_y[0:64, :], rcp_bc[:]
                    )

            # ---------------- Phase D: output projection ----------------
            with (
                tc.tile_pool(name="wo_pool", bufs=2) as wop,
                tc.tile_pool(name="obuf", bufs=2) as obuf,
                tc.tile_pool(name="po_pool", bufs=2, space="PSUM") as pop,
            ):
                for cc2 in range(NCC):
                    wo_t = wop.tile([P, NCC, P], F32R, name="wo_t")
                    nc.sync.dma_start(wo_t[:], wor_d[cc2])
                    pso = pop.tile([P, CH], F32, name="pso")
                    for cc in range(NCC):
                        nc.tensor.matmul(
                            pso[:],
                            lhsT=wo_t[:, cc, :],
                            rhs=yt_sb[cc][:],
                            start=(cc == 0), stop=(cc == NCC - 1),
                        )
                    ob = obuf.tile([P, CH], F32, name="ob")
                    nc.scalar.copy(ob[:], pso[:])
                    nc.sync.dma_start(out_d[cc2 * P:(cc2 + 1) * P, :], ob[:])

    nc.compile()
    return nc


def _host_mask():
    """(128, MW) bf16 band mask per core (varies only via left-edge validity)."""
    masks = []
    for core in range(NCORE):
        c = core % 4
        g0 = c * CH - HALO
        m = np.zeros((P, MW), np.float32)
        for jc in range(NJ):
            lo, hi, off = JW[jc]
            w = hi - lo
            j = (jc * P + np.arange(P))[:, None]          # local kv row
            i = (lo + np.arange(w))[None, :]              # local q row
            ok = (j >= i + 1) & (j <= i + WINDOW) & (g0 + j >= 0)
            m[:, off:off + w] = ok.astype(np.float32)
        masks.append(m.astype(ml_dtypes.bfloat16))
    return masks


_MASKS = _host_mask()
_NC_CACHE = {}


def _get_nc():
    if "nc" not in _NC_CACHE:
        _NC_CACHE["nc"] = build_nc()
    return _NC_CACHE["nc"]


def _in_maps(x, Wq, Wk, Wv, Wo, rope_cos, rope_sin):
    x = np.asarray(x, np.float32)
    cos = np.asarray(rope_cos, np.float32)[:, 0, :]   # (T, 32)
    sin = np.asarray(rope_sin, np.float32)[:, 0, :]

    def wr_cols(W):
        # [cc2, p, cc, m] = W[cc*128+p, cc2*128+m], contiguous per cc2
        return np.ascontiguousarray(
            np.asarray(W, np.float32).reshape(NCC, P, NCC, P).transpose(2, 1, 0, 3)
        )

    wqr = wr_cols(Wq)
    wkr = wr_cols(Wk)
    wor = wr_cols(Wo)
    wvr = np.ascontiguousarray(np.asarray(Wv, np.float32).reshape(NCC, P, C))

    maps = []
    for core in range(NCORE):
        b, c = divmod(core, 4)
        s = c * CH
        g0 = s - HALO
        xs = np.zeros((KT, C), np.float32)
        a0 = max(0, g0)
        xs[a0 - g0:s + CH - g0] = x[b, a0:s + CH]
        xT = np.ascontiguousarray(xs.T)

        csq = np.concatenate([cos[s:s + CH].T, sin[s:s + CH].T], 0)      # (64, 512)
        gidx = np.clip(g0 + np.arange(KT), 0, T - 1)
        csk = 2.0 * np.concatenate([cos[gidx].T, sin[gidx].T], 0)        # (64, KT)
        maps.append({
            "xT": xT,
            "wqr": wqr, "wkr": wkr, "wor": wor, "wvr": wvr,
            "csq": np.ascontiguousarray(np.tile(csq, (2, 1))),
            "csk": np.ascontiguousarray(np.tile(csk, (2, 1))),
            "mask": _MASKS[core],
        })
    return maps


def run(inputs, **kw):
    nc = _get_nc()
    maps = _in_maps(**inputs)
    res = run_bass_kernel_spmd(nc, maps, core_ids=list(range(NCORE)), **kw)
    out = np.zeros((B, T, C), np.float32)
    for core in range(NCORE):
        b, c = divmod(core, 4)
        s = c * CH
        out[b, s:s + CH, :] = res.results[core]["out"].T
    return out, res


def kernel(**inputs):
    out, _ = run(inputs)
    return out


if __name__ == "__main__":
    # graph-build smoke test
    nc = build_nc()
    print("build ok")


# revision 9
# speedup vs baseline: 1.3270x; 1.3270x over previous
"""Sliding-window attention (B=2,T=2048,C=1024,H=16,HD=64,WINDOW=524) on 8 trn2 cores.

Sharding: sequence-parallel. Core k = b*4+c owns query rows [c*512,(c+1)*512) of
batch b and receives x rows [c*512-524, c*512+512) (zero-padded outside the
sequence) so all of its attention windows are local. No collectives.

Per-core device pipeline (everything transposed so contractions land on the
partition axis, all matmul operands bf16, f32 accumulation):
  qT = Wq^T x^T (only own 512 rows), kT = Wk^T x^T (all 1152 local rows),
  RoPE folded into elementwise cos/sin scaling:
     scores q'.k' == (q * cs_i) . (2k * cs_j)  with cs = [cos;sin] per head,
  V computed in natural (t, c) layout with a ones column appended,
  S^T blocks (kv on partitions, q on free dim) matmul'd into two bank-aligned
  PSUM super-tiles per head, one exp per super-tile (P in bf16), one band-mask
  multiply per head, y^T (+ softmax denominator) = [V|1]^T @ P in PSUM,
  batched reciprocal over all 16 head denominators, normalize, Wo^T y^T.
"""

import os
import sys

import numpy as np

for _p in ("/opt/trn_rl_repo",):
    if _p not in sys.path and os.path.isdir(_p):
        sys.path.insert(0, _p)

import ml_dtypes

import concourse.bacc as bacc
import concourse.bass as bass
import concourse.mybir as mybir
from concourse.bass_utils import run_bass_kernel_spmd
from concourse.tile import TileContext

F32 = mybir.dt.float32
BF16 = mybir.dt.bfloat16
NPBF16 = ml_dtypes.bfloat16

B, T, C, H, HD = 2, 2048, 1024, 16, 64
WINDOW = 524
P = 128
CH = 512          # query rows per core
HALO = 524        # kv halo rows before the chunk
KT = 1152         # padded local kv length (1036 -> 9*128)
NJ = KT // P      # 9 j-chunks
NCC = C // P      # 8 contraction chunks
NCORE = 8

# Per-j-chunk query windows [lo, hi) in local query coords, 16-aligned lo.
JW = []
for _jc in range(NJ):
    _lo = max(0, P * _jc - 528)
    _hi = min(CH, P * _jc + P)
    JW.append((_lo, _hi))

# PSUM super-tile packing: each jc's scores window is placed bank-aligned
# (512-f32 banks) inside group G0 (3 banks) or G1 (3 banks).  pcol = column in
# the concatenated P/mask buffer [G0 | G1].
G0_OFF = {3: 0, 4: 512, 5: 1024}
G0_W = 1424
G1_OFF = {0: 0, 1: 128, 2: 512, 6: 1024, 7: 1296, 8: 1440}
G1_W = 1536
PW = G0_W + G1_W  # 2960
PCOL = {jc: off for jc, off in G0_OFF.items()}
PCOL.update({jc: G0_W + off for jc, off in G1_OFF.items()})

# k spans for the kT projection (rhs free width, psum bank limit 512)
KSPANS = [(0, 512), (512, 896), (896, 1152)]

JC_ORDER = [3, 4, 5, 0, 1, 2, 6, 7, 8]  # jc=3 first: its AV matmul covers [0,512)


def build_nc():
    nc = bacc.Bacc(None, target_bir_lowering=False)

    xT_d = nc.declare_dram_parameter("xT", [C, KT], BF16, isOutput=False)
    wqr_d = nc.declare_dram_parameter("wqr", [NCC, P, NCC, P], BF16, isOutput=False)
    wkr_d = nc.declare_dram_parameter("wkr", [NCC, P, NCC, P], BF16, isOutput=False)
    wor_d = nc.declare_dram_parameter("wor", [NCC, P, NCC, P], BF16, isOutput=False)
    wvr_d = nc.declare_dram_parameter("wvr", [NCC, P, C], BF16, isOutput=False)
    csq_d = nc.declare_dram_parameter("csq", [P, CH], F32, isOutput=False)
    csk_d = nc.declare_dram_parameter("csk", [P, KT], F32, isOutput=False)
    mask_d = nc.declare_dram_parameter("mask", [P, PW], BF16, isOutput=False)
    out_d = nc.declare_dram_parameter("out", [C, CH], F32, isOutput=True)

    Exp = mybir.ActivationFunctionType.Exp

    with TileContext(nc) as tc:
        with tc.tile_pool(name="persist", bufs=1) as pers:
            csq_sb = pers.tile([P, CH], F32, name="csq_sb")
            nc.sync.dma_start(csq_sb[:], csq_d[:])
            csk_sb = pers.tile([P, KT], F32, name="csk_sb")
            nc.sync.dma_start(csk_sb[:], csk_d[:])
            mask_sb = pers.tile([P, PW], BF16, name="mask_sb")
            nc.sync.dma_start(mask_sb[:], mask_d[:])

            qt_sb = []
            kt_sb = []
            yt_sb = []
            for cc2 in range(NCC):
                qt_sb.append(pers.tile([P, CH], BF16, name=f"qt{cc2}"))
                kt_sb.append(pers.tile([P, KT], BF16, name=f"kt{cc2}"))
                yt_sb.append(pers.tile([P, CH], BF16, name=f"yt{cc2}"))
            v_sb = pers.tile([P, NJ, H, 65], BF16, name="v_sb")
            nc.vector.memset(v_sb[:, :, :, 64], 1.0)
            yhat_sb = pers.tile([P, NCC, CH], F32, name="yhat_sb")
            den_sb = pers.tile([32, CH], F32, name="den_sb")
            rcp_sb = pers.tile([32, CH], F32, name="rcp_sb")
            nc.vector.memset(den_sb[:], 1.0)

            # ---------------- Phase B: projections ----------------
            with (
                tc.tile_pool(name="xw", bufs=1) as xw,
                tc.tile_pool(name="wv_pool", bufs=1) as wvp,
                tc.tile_pool(name="ws", bufs=3) as ws,
                tc.tile_pool(name="psb", bufs=2, space="PSUM") as psb,
                tc.tile_pool(name="psv_pool", bufs=2, space="PSUM") as psvp,
            ):
                xT_sb = []
                for cc in range(NCC):
                    xt = xw.tile([P, KT], BF16, name=f"xt{cc}")
                    nc.sync.dma_start(xt[:], xT_d[cc * P:(cc + 1) * P, :])
                    xT_sb.append(xt)
                wv_sb = []
                for cc in range(NCC):
                    wvt = wvp.tile([P, C], BF16, name=f"wvt{cc}")
                    nc.sync.dma_start(wvt[:], wvr_d[cc])
                    wv_sb.append(wvt)

                for cc2 in range(NCC):
                    wq_t = ws.tile([P, NCC, P], BF16, name="wq_t")
                    nc.sync.dma_start(wq_t[:], wqr_d[cc2])
                    wk_t = ws.tile([P, NCC, P], BF16, name="wk_t")
                    nc.sync.dma_start(wk_t[:], wkr_d[cc2])

                    psq = psb.tile([P, CH], F32, name="psq")
                    for cc in range(NCC):
                        nc.tensor.matmul(
                            psq[:],
                            lhsT=wq_t[:, cc, :],
                            rhs=xT_sb[cc][:, HALO:HALO + CH],
                            start=(cc == 0), stop=(cc == NCC - 1),
                        )
                    nc.vector.tensor_mul(qt_sb[cc2][:], psq[:], csq_sb[:])

                    for (a, b) in KSPANS:
                        psk = psb.tile([P, 512], F32, name="psk")
                        for cc in range(NCC):
                            nc.tensor.matmul(
                                psk[:, :b - a],
                                lhsT=wk_t[:, cc, :],
                                rhs=xT_sb[cc][:, a:b],
                                start=(cc == 0), stop=(cc == NCC - 1),
                            )
                        nc.vector.tensor_mul(
                            kt_sb[cc2][:, a:b], psk[:, :b - a], csk_sb[:, a:b]
                        )

                # V in natural layout: lhsT = xT tile (c, t-block), rhs = Wv rows
                for tb in range(NJ):
                    for half in range(2):
                        psv = psvp.tile([P, 512], F32, name="psv")
                        for cc in range(NCC):
                            nc.tensor.matmul(
                                psv[:],
                                lhsT=xT_sb[cc][:, tb * P:(tb + 1) * P],
                                rhs=wv_sb[cc][:, half * 512:(half + 1) * 512],
                                start=(cc == 0), stop=(cc == NCC - 1),
                            )
                        nc.scalar.copy(
                            v_sb[:, tb, half * 8:(half + 1) * 8, 0:64],
                            psv[:].rearrange("p (h d) -> p h d", h=8),
                        )

            # ---------------- Phase C: attention ----------------
            with (
                tc.tile_pool(name="ppool", bufs=2) as ppool,
                tc.tile_pool(name="rpool", bufs=2) as rpool,
                tc.tile_pool(name="pst_pool", bufs=1, space="PSUM") as pstp,
                tc.tile_pool(name="pay_pool", bufs=2, space="PSUM") as payp,
            ):
                for h in range(H):
                    cc2, po = h // 2, (h % 2) * 64
                    ps_g0 = pstp.tile([P, G0_W], F32, name="ps_g0")
                    ps_g1 = pstp.tile([P, G1_W], F32, name="ps_g1")
                    ps_y = payp.tile([P, CH], F32, name="ps_y")
                    P_t = ppool.tile([P, PW], BF16, name="P_t")
                    for jc in JC_ORDER:
                        lo, hi = JW[jc]
                        w = hi - lo
                        if jc in G0_OFF:
                            dst = ps_g0[:, G0_OFF[jc]:G0_OFF[jc] + w]
                        else:
                            dst = ps_g1[:, G1_OFF[jc]:G1_OFF[jc] + w]
                        nc.tensor.matmul(
                            dst,
                            lhsT=kt_sb[cc2][po:po + 64, jc * P:(jc + 1) * P],
                            rhs=qt_sb[cc2][po:po + 64, lo:hi],
                            start=True, stop=True,
                        )
                    nc.scalar.activation(P_t[:, 0:G0_W], ps_g0[:], Exp, scale=0.125)
                    nc.scalar.activation(P_t[:, G0_W:PW], ps_g1[:], Exp, scale=0.125)
                    nc.vector.tensor_mul(P_t[:], P_t[:], mask_sb[:])
                    for idx, jc in enumerate(JC_ORDER):
                        lo, hi = JW[jc]
                        w = hi - lo
                        pc = PCOL[jc]
                        nc.tensor.matmul(
                            ps_y[0:65, lo:hi],
                            lhsT=v_sb[:, jc, h, :],
                            rhs=P_t[:, pc:pc + w],
                            start=(idx == 0), stop=(idx == NJ - 1),
                        )
                    nc.scalar.copy(yhat_sb[po:po + 64, cc2, :], ps_y[0:64, :])
                    dtmp = rpool.tile([1, CH], F32, name="dtmp")
                    nc.vector.tensor_copy(dtmp[:], ps_y[64:65, :])
                    nc.sync.dma_start(den_sb[h:h + 1, :], dtmp[:])

                nc.vector.reciprocal(rcp_sb[:], den_sb[:])
                for h in range(H):
                    cc2, po = h // 2, (h % 2) * 64
                    rtmp = rpool.tile([1, CH], F32, name="rtmp")
                    nc.sync.dma_start(rtmp[:], rcp_sb[h:h + 1, :])
                    rcp_bc = rpool.tile([P, CH], F32, name="rcp_bc")
                    nc.gpsimd.partition_broadcast(rcp_bc[:], rtmp[:])
                    nc.vector.tensor_mul(
                        yt_sb[cc2][po:po + 64, :], yhat_sb[po:po + 64, cc2, :],
                        rcp_bc[po:po + 64, :],
                    )

            # ---------------- Phase D: output projection ----------------
            with (
                tc.tile_pool(name="wo_pool", bufs=2) as wop,
                tc.tile_pool(name="obuf", bufs=2) as obuf,
                tc.tile_pool(name="po_pool", bufs=2, space="PSUM") as pop,
            ):
                for cc2 in range(NCC):
                    wo_t = wop.tile([P, NCC, P], BF16, name="wo_t")
                    nc.sync.dma_start(wo_t[:], wor_d[cc2])
                    pso = pop.tile([P, CH], F32, name="pso")
                    for cc in range(NCC):
                        nc.tensor.matmul(
                            pso[:],
                            lhsT=wo_t[:, cc, :],
                            rhs=yt_sb[cc][:],
                            start=(cc == 0), stop=(cc == NCC - 1),
                        )
                    ob = obuf.tile([P, CH], F32, name="ob")
                    nc.scalar.copy(ob[:], pso[:])
                    nc.sync.dma_start(out_d[cc2 * P:(cc2 + 1) * P, :], ob[:])

    nc.compile()
    return nc


def _host_mask():
    """(128, PW) bf16 band mask per core; 0 in pack holes and outside band."""
    masks = []
    for core in range(NCORE):
        c = core % 4
        g0 = c * CH - HALO
        m = np.zeros((P, PW), np.float32)
        for jc in range(NJ):
            lo, hi = JW[jc]
            w = hi - lo
            j = (jc * P + np.arange(P))[:, None]          # local kv row
            i = (lo + np.arange(w))[None, :]              # local q row
            ok = (j >= i + 1) & (j <= i + WINDOW) & (g0 + j >= 0)
            pc = PCOL[jc]
            m[:, pc:pc + w] = ok.astype(np.float32)
        masks.append(m.astype(NPBF16))
    return masks


_MASKS = _host_mask()
_NC_CACHE = {}


def _get_nc():
    if "nc" not in _NC_CACHE:
        _NC_CACHE["nc"] = build_nc()
    return _NC_CACHE["nc"]


def _in_maps(x, Wq, Wk, Wv, Wo, rope_cos, rope_sin):
    x = np.asarray(x, np.float32)
    cos = np.asarray(rope_cos, np.float32)[:, 0, :]   # (T, 32)
    sin = np.asarray(rope_sin, np.float32)[:, 0, :]

    def wr_cols(W):
        # [cc2, p, cc, m] = W[cc*128+p, cc2*128+m], contiguous per cc2
        return np.ascontiguousarray(
            np.asarray(W, np.float32).reshape(NCC, P, NCC, P).transpose(2, 1, 0, 3)
        ).astype(NPBF16)

    wqr = wr_cols(Wq)
    wkr = wr_cols(Wk)
    wor = wr_cols(Wo)
    wvr = np.asarray(Wv, np.float32).reshape(NCC, P, C).astype(NPBF16)

    maps = []
    for core in range(NCORE):
        b, c = divmod(core, 4)
        s = c * CH
        g0 = s - HALO
        xs = np.zeros((KT, C), np.float32)
        a0 = max(0, g0)
        xs[a0 - g0:s + CH - g0] = x[b, a0:s + CH]
        xT = np.ascontiguousarray(xs.T).astype(NPBF16)

        csq = np.concatenate([cos[s:s + CH].T, sin[s:s + CH].T], 0)      # (64, 512)
        gidx = np.clip(g0 + np.arange(KT), 0, T - 1)
        csk = 2.0 * np.concatenate([cos[gidx].T, sin[gidx].T], 0)        # (64, KT)
        maps.append({
            "xT": xT,
            "wqr": wqr, "wkr": wkr, "wor": wor, "wvr": wvr,
            "csq": np.ascontiguousarray(np.tile(csq, (2, 1))),
            "csk": np.ascontiguousarray(np.tile(csk, (2, 1))),
            "mask": _MASKS[core],
        })
    return maps


def run(inputs, **kw):
    nc = _get_nc()
    maps = _in_maps(**inputs)
    res = run_bass_kernel_spmd(nc, maps, core_ids=list(range(NCORE)), **kw)
    out = np.zeros((B, T, C), np.float32)
    for core in range(NCORE):
        b, c = divmod(core, 4)
        s = c * CH
        out[b, s:s + CH, :] = res.results[core]["out"].T
    return out, res


def kernel(**inputs):
    out, _ = run(inputs)
    return out


if __name__ == "__main__":
    # graph-build smoke test
    nc = build_nc()
    print("build ok")


# revision 11
# speedup vs baseline: 1.3910x; 1.0483x over previous
"""Sliding-window attention (B=2,T=2048,C=1024,H=16,HD=64,WINDOW=524) on 8 trn2 cores.

Sharding: sequence-parallel. Core k = b*4+c owns query rows [c*512,(c+1)*512) of
batch b and receives x rows [c*512-524, c*512+512) (zero-padded outside the
sequence) so all of its attention windows are local. No collectives.

Per-core device pipeline (everything transposed so contractions land on the
partition axis, all matmul operands bf16, f32 accumulation):
  qT = Wq^T x^T (only own 512 rows), kT = Wk^T x^T (all 1152 local rows),
  RoPE folded into elementwise cos/sin scaling:
     scores q'.k' == (q * cs_i) . (2k * cs_j)  with cs = [cos;sin] per head,
  V computed in natural (t, c) layout with a ones column appended,
  S^T blocks (kv on partitions, q on free dim) matmul'd into two bank-aligned
  PSUM super-tiles per head, one exp per super-tile (P in bf16), one band-mask
  multiply per head, y^T (+ softmax denominator) = [V|1]^T @ P in PSUM,
  batched reciprocal over all 16 head denominators, normalize, Wo^T y^T.
"""

import os
import sys

import numpy as np

for _p in ("/opt/trn_rl_repo",):
    if _p not in sys.path and os.path.isdir(_p):
        sys.path.insert(0, _p)

import ml_dtypes

import concourse.bacc as bacc
import concourse.bass as bass
import concourse.mybir as mybir
from concourse.bass_utils import run_bass_kernel_spmd
from concourse.tile import TileContext

F32 = mybir.dt.float32
BF16 = mybir.dt.bfloat16
NPBF16 = ml_dtypes.bfloat16

B, T, C, H, HD = 2, 2048, 1024, 16, 64
WINDOW = 524
P = 128
CH = 512          # query rows per core
HALO = 524        # kv halo rows before the chunk
KT = 1040         # padded local kv length (1036 -> 1040; last j-chunk is 16 rows)
NJ = 9            # j-chunks: 8 full + one 16-row tail
NCC = C // P      # 8 contraction chunks
NCORE = 8

# Per-j-chunk query windows [lo, hi) in local query coords, 16-aligned lo.
JW = []
for _jc in range(NJ):
    _lo = max(0, P * _jc - 528)
    _hi = min(CH, P * _jc + P)
    JW.append((_lo, _hi))

# PSUM super-tile packing: each jc's scores window is placed bank-aligned
# (512-f32 banks) inside group G0 (3 banks) or G1 (3 banks).  pcol = column in
# the concatenated P/mask buffer [G0 | G1].
G0_OFF = {3: 0, 4: 512, 5: 1024}
G0_W = 1424
G1_OFF = {0: 0, 1: 128, 2: 512, 6: 1024, 7: 1296, 8: 1440}
G1_W = 1536
PW = G0_W + G1_W  # 2960
PCOL = {jc: off for jc, off in G0_OFF.items()}
PCOL.update({jc: G0_W + off for jc, off in G1_OFF.items()})

# k spans for the kT projection (rhs free width, psum bank limit 512)
KSPANS = [(0, 512), (512, 784), (784, 1040)]

JC_ORDER = [3, 4, 5, 0, 1, 2, 6, 7, 8]  # jc=3 first: its AV matmul covers [0,512)


def build_nc():
    nc = bacc.Bacc(None, target_bir_lowering=False)

    xT_d = nc.declare_dram_parameter("xT", [C, KT], BF16, isOutput=False)
    wqr_d = nc.declare_dram_parameter("wqr", [NCC, P, NCC, P], BF16, isOutput=False)
    wkr_d = nc.declare_dram_parameter("wkr", [NCC, P, NCC, P], BF16, isOutput=False)
    wor_d = nc.declare_dram_parameter("wor", [NCC, P, NCC, P], BF16, isOutput=False)
    wvr_d = nc.declare_dram_parameter("wvr", [NCC, P, C], BF16, isOutput=False)
    csq_d = nc.declare_dram_parameter("csq", [P, CH], F32, isOutput=False)
    csk_d = nc.declare_dram_parameter("csk", [P, KT], F32, isOutput=False)
    mask_d = nc.declare_dram_parameter("mask", [P, PW], BF16, isOutput=False)
    out_d = nc.declare_dram_parameter("out", [C, CH], F32, isOutput=True)

    Exp = mybir.ActivationFunctionType.Exp

    with TileContext(nc) as tc:
        with tc.tile_pool(name="persist", bufs=1) as pers:
            csq_sb = pers.tile([P, CH], F32, name="csq_sb")
            nc.sync.dma_start(csq_sb[:], csq_d[:])
            csk_sb = pers.tile([P, KT], F32, name="csk_sb")
            nc.sync.dma_start(csk_sb[:], csk_d[:])
            mask_sb = pers.tile([P, PW], BF16, name="mask_sb")
            nc.sync.dma_start(mask_sb[:], mask_d[:])

            qt_sb = []
            kt_sb = []
            yt_sb = []
            for cc2 in range(NCC):
                qt_sb.append(pers.tile([P, CH], BF16, name=f"qt{cc2}"))
                kt_sb.append(pers.tile([P, KT], BF16, name=f"kt{cc2}"))
                yt_sb.append(pers.tile([P, CH], BF16, name=f"yt{cc2}"))
            v_sb = pers.tile([P, NJ, H, 65], BF16, name="v_sb")
            nc.vector.memset(v_sb[:, :, :, 64], 1.0)
            wo_pre = []
            for cc2 in range(NCC):
                wot = pers.tile([P, NCC, P], BF16, name=f"wot{cc2}")
                nc.sync.dma_start(wot[:], wor_d[cc2])
                wo_pre.append(wot)
            yhat_sb = pers.tile([P, NCC, CH], F32, name="yhat_sb")
            den_g = []
            rcp_g = []
            for g in range(4):
                den_g.append(pers.tile([4, CH], F32, name=f"den{g}"))
                rcp_g.append(pers.tile([4, CH], F32, name=f"rcp{g}"))

            # ---------------- Phase B: projections ----------------
            with (
                tc.tile_pool(name="xw", bufs=1) as xw,
                tc.tile_pool(name="wv_pool", bufs=1) as wvp,
                tc.tile_pool(name="ws", bufs=3) as ws,
                tc.tile_pool(name="psb", bufs=2, space="PSUM") as psb,
                tc.tile_pool(name="psv_pool", bufs=2, space="PSUM") as psvp,
            ):
                xT_sb = []
                for cc in range(NCC):
                    xt = xw.tile([P, KT], BF16, name=f"xt{cc}")
                    nc.sync.dma_start(xt[:], xT_d[cc * P:(cc + 1) * P, :])
                    xT_sb.append(xt)
                wv_sb = []
                for cc in range(NCC):
                    wvt = wvp.tile([P, C], BF16, name=f"wvt{cc}")
                    nc.sync.dma_start(wvt[:], wvr_d[cc])
                    wv_sb.append(wvt)

                for cc2 in range(NCC):
                    wq_t = ws.tile([P, NCC, P], BF16, name="wq_t")
                    nc.sync.dma_start(wq_t[:], wqr_d[cc2])
                    wk_t = ws.tile([P, NCC, P], BF16, name="wk_t")
                    nc.sync.dma_start(wk_t[:], wkr_d[cc2])

                    psq = psb.tile([P, CH], F32, name="psq")
                    for cc in range(NCC):
                        nc.tensor.matmul(
                            psq[:],
                            lhsT=wq_t[:, cc, :],
                            rhs=xT_sb[cc][:, HALO:HALO + CH],
                            start=(cc == 0), stop=(cc == NCC - 1),
                        )
                    nc.vector.tensor_mul(qt_sb[cc2][:], psq[:], csq_sb[:])

                    for (a, b) in KSPANS:
                        psk = psb.tile([P, 512], F32, name="psk")
                        for cc in range(NCC):
                            nc.tensor.matmul(
                                psk[:, :b - a],
                                lhsT=wk_t[:, cc, :],
                                rhs=xT_sb[cc][:, a:b],
                                start=(cc == 0), stop=(cc == NCC - 1),
                            )
                        nc.vector.tensor_mul(
                            kt_sb[cc2][:, a:b], psk[:, :b - a], csk_sb[:, a:b]
                        )

                # V in natural layout: lhsT = xT tile (c, t-block), rhs = Wv rows
                for tb in range(NJ):
                    tp = min(P, KT - tb * P)   # 128, or 16 for the tail block
                    for half in range(2):
                        psv = psvp.tile([P, 512], F32, name="psv")
                        for cc in range(NCC):
                            nc.tensor.matmul(
                                psv[0:tp, :],
                                lhsT=xT_sb[cc][:, tb * P:tb * P + tp],
                                rhs=wv_sb[cc][:, half * 512:(half + 1) * 512],
                                start=(cc == 0), stop=(cc == NCC - 1),
                            )
                        nc.scalar.copy(
                            v_sb[0:tp, tb, half * 8:(half + 1) * 8, 0:64],
                            psv[0:tp, :].rearrange("p (h d) -> p h d", h=8),
                        )

            # ---------------- Phase C: attention ----------------
            with (
                tc.tile_pool(name="ppool", bufs=2) as ppool,
                tc.tile_pool(name="rpool", bufs=2) as rpool,
                tc.tile_pool(name="pst_pool", bufs=1, space="PSUM") as pstp,
                tc.tile_pool(name="pay_pool", bufs=2, space="PSUM") as payp,
            ):
                for g in range(4):
                    for h in range(4 * g, 4 * g + 4):
                        cc2, po = h // 2, (h % 2) * 64
                        ps_g0 = pstp.tile([P, G0_W], F32, name="ps_g0")
                        ps_g1 = pstp.tile([P, G1_W], F32, name="ps_g1")
                        ps_y = payp.tile([P, CH], F32, name="ps_y")
                        P_t = ppool.tile([P, PW], BF16, name="P_t")
                        for jc in JC_ORDER:
                            lo, hi = JW[jc]
                            w = hi - lo
                            jp = min(P, KT - jc * P)
                            if jc in G0_OFF:
                                dst = ps_g0[0:jp, G0_OFF[jc]:G0_OFF[jc] + w]
                            else:
                                dst = ps_g1[0:jp, G1_OFF[jc]:G1_OFF[jc] + w]
                            nc.tensor.matmul(
                                dst,
                                lhsT=kt_sb[cc2][po:po + 64, jc * P:jc * P + jp],
                                rhs=qt_sb[cc2][po:po + 64, lo:hi],
                                start=True, stop=True,
                            )
                        nc.scalar.activation(P_t[:, 0:G0_W], ps_g0[:], Exp, scale=0.125)
                        nc.scalar.activation(P_t[:, G0_W:PW], ps_g1[:], Exp, scale=0.125)
                        nc.vector.tensor_mul(P_t[:], P_t[:], mask_sb[:])
                        for idx, jc in enumerate(JC_ORDER):
                            lo, hi = JW[jc]
                            w = hi - lo
                            pc = PCOL[jc]
                            jp = min(P, KT - jc * P)
                            nc.tensor.matmul(
                                ps_y[0:65, lo:hi],
                                lhsT=v_sb[0:jp, jc, h, :],
                                rhs=P_t[0:jp, pc:pc + w],
                                start=(idx == 0), stop=(idx == NJ - 1),
                            )
                        nc.scalar.copy(yhat_sb[po:po + 64, cc2, :], ps_y[0:64, :])
                        dtmp = rpool.tile([1, CH], F32, name="dtmp")
                        nc.vector.tensor_copy(dtmp[:], ps_y[64:65, :])
                        nc.sync.dma_start(den_g[g][h % 4:h % 4 + 1, :], dtmp[:])

                    nc.vector.reciprocal(rcp_g[g][:], den_g[g][:])
                    for h in range(4 * g, 4 * g + 4):
                        cc2, po = h // 2, (h % 2) * 64
                        rtmp = rpool.tile([1, CH], F32, name="rtmp")
                        nc.sync.dma_start(rtmp[:], rcp_g[g][h % 4:h % 4 + 1, :])
                        rcp_bc = rpool.tile([P, CH], F32, name="rcp_bc")
                        nc.gpsimd.partition_broadcast(rcp_bc[:], rtmp[:])
                        nc.vector.tensor_mul(
                            yt_sb[cc2][po:po + 64, :], yhat_sb[po:po + 64, cc2, :],
                            rcp_bc[po:po + 64, :],
                        )

            # ---------------- Phase D: output projection ----------------
            with (
                tc.tile_pool(name="obuf", bufs=2) as obuf,
                tc.tile_pool(name="po_pool", bufs=2, space="PSUM") as pop,
            ):
                for cc2 in range(NCC):
                    wo_t = wo_pre[cc2]
                    pso = pop.tile([P, CH], F32, name="pso")
                    for cc in range(NCC):
                        nc.tensor.matmul(
                            pso[:],
                            lhsT=wo_t[:, cc, :],
                            rhs=yt_sb[cc][:],
                            start=(cc == 0), stop=(cc == NCC - 1),
                        )
                    ob = obuf.tile([P, CH], F32, name="ob")
                    nc.scalar.copy(ob[:], pso[:])
                    nc.sync.dma_start(out_d[cc2 * P:(cc2 + 1) * P, :], ob[:])

    nc.compile()
    return nc


def _host_mask():
    """(128, PW) bf16 band mask per core; 0 in pack holes and outside band."""
    masks = []
    for core in range(NCORE):
        c = core % 4
        g0 = c * CH - HALO
        m = np.zeros((P, PW), np.float32)
        for jc in range(NJ):
            lo, hi = JW[jc]
            w = hi - lo
            j = (jc * P + np.arange(P))[:, None]          # local kv row
            i = (lo + np.arange(w))[None, :]              # local q row
            ok = (j >= i + 1) & (j <= i + WINDOW) & (g0 + j >= 0)
            pc = PCOL[jc]
            m[:, pc:pc + w] = ok.astype(np.float32)
        masks.append(m.astype(NPBF16))
    return masks


_MASKS = _host_mask()
_NC_CACHE = {}


def _get_nc():
    if "nc" not in _NC_CACHE:
        _NC_CACHE["nc"] = build_nc()
    return _NC_CACHE["nc"]


def _in_maps(x, Wq, Wk, Wv, Wo, rope_cos, rope_sin):
    x = np.asarray(x, np.float32)
    cos = np.asarray(rope_cos, np.float32)[:, 0, :]   # (T, 32)
    sin = np.asarray(rope_sin, np.float32)[:, 0, :]

    def wr_cols(W):
        # [cc2, p, cc, m] = W[cc*128+p, cc2*128+m], contiguous per cc2
        return np.ascontiguousarray(
            np.asarray(W, np.float32).reshape(NCC, P, NCC, P).transpose(2, 1, 0, 3)
        ).astype(NPBF16)

    wqr = wr_cols(Wq)
    wkr = wr_cols(Wk)
    wor = wr_cols(Wo)
    wvr = np.asarray(Wv, np.float32).reshape(NCC, P, C).astype(NPBF16)

    maps = []
    for core in range(NCORE):
        b, c = divmod(core, 4)
        s = c * CH
        g0 = s - HALO
        xs = np.zeros((KT, C), np.float32)
        a0 = max(0, g0)
        xs[a0 - g0:s + CH - g0] = x[b, a0:s + CH]
        xT = np.ascontiguousarray(xs.T).astype(NPBF16)

        csq = np.concatenate([cos[s:s + CH].T, sin[s:s + CH].T], 0)      # (64, 512)
        gidx = np.clip(g0 + np.arange(KT), 0, T - 1)
        csk = 2.0 * np.concatenate([cos[gidx].T, sin[gidx].T], 0)        # (64, KT)
        maps.append({
            "xT": xT,
            "wqr": wqr, "wkr": wkr, "wor": wor, "wvr": wvr,
            "csq": np.ascontiguousarray(np.tile(csq, (2, 1))),
            "csk": np.ascontiguousarray(np.tile(csk, (2, 1))),
            "mask": _MASKS[core],
        })
    return maps


def run(inputs, **kw):
    nc = _get_nc()
    maps = _in_maps(**inputs)
    res = run_bass_kernel_spmd(nc, maps, core_ids=list(range(NCORE)), **kw)
    out = np.zeros((B, T, C), np.float32)
    for core in range(NCORE):
        b, c = divmod(core, 4)
        s = c * CH
        out[b, s:s + CH, :] = res.results[core]["out"].T
    return out, res


def kernel(**inputs):
    out, _ = run(inputs)
    return out


if __name__ == "__main__":
    # graph-build smoke test
    nc = build_nc()
    print("build ok")
